# revision 13
# baseline (speedup 1.0000x reference)
"""Involution-style per-pixel depthwise 3x3 conv on 8 trn2 NeuronCores.

out[n,c,h,w] = sum_{k=0..8} w[n,c,k,h,w] * x_pad[n,c,h+k//3,w+k%3]  (pad=1)

Sharding: pure data parallel over N=8 -> one sample per core.
Per core: channels C=128 = SBUF partition dim; free dim = H*W pixels.

v2 design (bf16 compute; harness gate is rel<2e-2, bf16 lands ~1e-3):
- The v1 kernel was DVE-bound: fp32 tensor_tensor runs at 1 elem/cycle,
  so 9 muls + ~4.5 adds per pixel = ~139 us DVE busy vs a ~146 us HBM
  stream (52 MB @ ~358 GB/s).  DMA engines idled 28% waiting on slab
  buffer reuse.
- Weights are cast fp32->bf16 DURING the load DMA (SWDGE/gpsimd path
  does inline dtype conversion; HBM read bytes unchanged, zero engine
  cost).  bf16 tensor_tensor runs at 2 elem/cycle -> products ~50 us.
- DVE 2x mode needs 4B-aligned step-1 operands, but the 3x3 taps read x
  at column offsets -1/0/+1 (odd/even element parity).  So x lives in
  SBUF twice in bf16: xg_even (guard 96) serves the dj=0 taps, xg_odd
  (guard 97) serves dj=+-1; both windows land on even element offsets.
  One [2,2]-strided AP covers the +-1 tap pair in a single DVE op.
- The 9-tap sum runs entirely on the otherwise-idle TensorE: bf16
  identity matmuls accumulate the product planes into fp32 PSUM
  (exact), ScalarE evacuates PSUM->SBUF fp32, HWDGE stores.  No DVE
  adds at all.
- Weight slab border columns (the column-wrap pixels of the dj=+-1
  taps) are zeroed on the VECTOR engine, not gpsimd: a gpsimd memset
  waiting on its slab DMA would stall the gpsimd sequencer and with it
  all later SWDGE descriptor emission -- the weight stream itself.
- One SWDGE queue carries everything that reads HBM: weight chunks of
  (16,16,16,16,16,8,6,2) rows x 3 tap-groups (cast loads, 14 slab
  buffers ~= 4.7 chunks of lookahead) with x cast-loads interleaved
  just-in-time.  The measured stream is gapless at ~26 GB/s per SDMA
  engine.  Stores ride the ACT HWDGE ring.  Tiny final chunks keep the
  exposed mul->matmul->evac->store tail after the last weight byte to
  ~2.5 us; the first chunk's DMAs are emitted before the identity/guard
  setup so the stream starts as early as the ~5.5 us preamble allows.
- Measured: ~139 us best (fully contended HBM reps land 145-165); DVE
  ~48 us busy, PE ~52, ScalarE ~20 -- all far off the critical path.
"""

import numpy as np

import concourse.bass as bass
import concourse.mybir as mybir
from concourse.bass_utils import run_bass_kernel_spmd
from concourse.masks import make_identity
from concourse.tile import TileContext

N_CORES = 8
C, H, W = 128, 96, 96
HW = H * W
KW = 3

F32 = mybir.dt.float32
BF16 = mybir.dt.bfloat16

# row-groups: (name, first tap k0, row shift di)
GROUPS = (("mid", 3, 0), ("top", 0, -1), ("bot", 6, 1))

# guarded x layouts (bf16):
#   xg_even: [96 zeros | x | 96 zeros]  -> dj=0  windows at even offsets
#   xg_odd:  [97 zeros | x | 97 zeros]  -> dj=+-1 windows at even offsets
GE = 96
GO = 97
GXE = HW + 2 * GE
GXO = HW + 2 * GO

# weight DMA row-chunks; each chunk is 3 slab DMAs (one per tap group).
# Small final chunks: the last chunk's mul->matmul->evac->store chain is
# exposed after the final weight byte lands, so keep it tiny.
CHUNKS = (16, 16, 16, 16, 16, 8, 6, 2)
assert sum(CHUNKS) == H
MAXR = max(CHUNKS)
SLAB = MAXR * W          # slab tile elems per tap per partition

# x load chunks (SWDGE cast loads interleaved with the weight chunks)
XCH = 24                 # rows per x chunk
N_XCH = H // XCH

SUBMAX = 16              # compute sub-stripe rows (PSUM sizing)
SL = SUBMAX * W


def _build() -> bass.Bass:
    nc = bass.Bass()
    x_d = nc.dram_tensor("x", [C, HW], F32, kind="ExternalInput")
    w_d = nc.dram_tensor("w", [C * KW * KW, HW], F32, kind="ExternalInput")
    o_d = nc.dram_tensor("out", [C, HW], F32, kind="ExternalOutput")

    w_v = w_d[:].rearrange("(c k) m -> c k m", k=KW * KW)

    with TileContext(nc) as tc:
        with (
            tc.tile_pool(name="px", bufs=1) as px,
            tc.tile_pool(name="pw", bufs=14) as pw,
            tc.tile_pool(name="pg", bufs=2) as pg,
            tc.tile_pool(name="pp", bufs=2, space="PSUM") as pp,
        ):
            ident_f = px.tile([C, C], F32)
            ident = px.tile([C, C], BF16)
            xg_e = px.tile([C, GXE], BF16)
            xg_o = px.tile([C, GXO], BF16)

            def load_x_chunk(j):
                """SWDGE cast-load of x rows [24j, 24j+24) into xg_even;
                ScalarE copies the chunk into the odd-parity buffer."""
                r0, n = j * XCH, XCH * W
                nc.gpsimd.dma_start(
                    out=xg_e[:, GE + r0 * W : GE + r0 * W + n],
                    in_=x_d[:, r0 * W : r0 * W + n],
                )
                nc.scalar.copy(
                    out=xg_o[:, GO + r0 * W : GO + r0 * W + n],
                    in_=xg_e[:, GE + r0 * W : GE + r0 * W + n],
                )

            def slab_pair_ap(slab, o, n):
                """[C, 2, n] AP over slab taps {0,2} at in-slab offset o."""
                base = slab[:, 0, o : o + n]
                ap = [list(p) for p in base.ap]
                return bass.AP(base.tensor, base.offset, [ap[0], [2 * SLAB, 2], [1, n]])

            def x_pair_ap(r0, di, n):
                """[C, 2, n] AP over xg_odd: dj=-1 and dj=+1 windows."""
                off = GO + (r0 + di) * W - 1
                base = xg_o[:, off : off + n]
                ap = [list(p) for p in base.ap]
                return bass.AP(base.tensor, base.offset, [ap[0], [2, 2], [1, n]])

            # x chunk j covers rows [24j, 24j+24); weight chunk ci's bot
            # group reads x rows up to r0+rr (inclusive), so chunk j loads
            # just before the first weight chunk that needs it:
            #   c0 rows 0-15 needs x<=16 -> x0; c1 (16-31) needs 32 -> x1;
            #   c2 (32-47) needs 48 -> x2; c4 (64-79) needs 80 -> x3.
            x_before = {0: 0, 1: 1, 2: 2, 4: 3}

            def issue_chunk_dmas(ci, r0, rr):
                """SWDGE cast loads for one row-chunk (3 tap-group slabs)."""
                cn = rr * W
                slabs = {}
                for gname, k0, di in GROUPS:
                    slab = pw.tile([C, KW, SLAB], BF16, tag="w", name=f"w_{gname}_{ci}")
                    nc.gpsimd.dma_start(
                        out=slab[:, :, 0:cn],
                        in_=w_v[:, k0 : k0 + KW, r0 * W : (r0 + rr) * W],
                    )
                    slabs[gname] = slab
                return slabs

            # startup-critical: get the first x chunk + weight chunk onto
            # the SWDGE queue before the identity/guard setup ops occupy
            # the gpsimd sequencer.
            load_x_chunk(x_before[0])
            slabs0 = issue_chunk_dmas(0, 0, CHUNKS[0])

            make_identity(nc, ident_f)
            nc.vector.tensor_copy(out=ident[:, :], in_=ident_f[:, :])
            # zero the vertical-padding guards once -- on DVE (idle until
            # the first slab lands), keeping the gpsimd queue pure SWDGE
            nc.vector.memset(xg_e[:, 0:GE], 0.0)
            nc.vector.memset(xg_e[:, GE + HW : GXE], 0.0)
            nc.vector.memset(xg_o[:, 0:GO], 0.0)
            nc.vector.memset(xg_o[:, GO + HW : GXO], 0.0)

            r0 = 0
            for ci, rr in enumerate(CHUNKS):
                cn = rr * W
                if ci == 0:
                    slabs = slabs0
                else:
                    if ci in x_before:
                        load_x_chunk(x_before[ci])
                    slabs = issue_chunk_dmas(ci, r0, rr)
                for gname, k0, di in GROUPS:
                    # zero border weight columns on DVE (kills the
                    # column-wrap reads of the dj=+-1 taps)
                    sr = slabs[gname].rearrange("p k (h w) -> p k h w", w=W)
                    nc.vector.memset(sr[:, 0, 0:rr, 0:1], 0.0)
                    nc.vector.memset(sr[:, 2, 0:rr, W - 1 : W], 0.0)

                # compute sub-stripes within this chunk
                o = 0
                while o < cn:
                    n = min(SUBMAX * W, cn - o)
                    sr0 = r0 + o // W
                    for gname, k0, di in GROUPS:
                        slab = slabs[gname]
                        # dj=-1,+1 pair: one 2x-mode TT over [C,2,n]
                        wpair = slab_pair_ap(slab, o, n)
                        nc.vector.tensor_mul(
                            out=wpair, in0=wpair, in1=x_pair_ap(sr0, di, n)
                        )
                        # dj=0 center tap from xg_even
                        xoff = GE + (sr0 + di) * W
                        nc.vector.tensor_mul(
                            out=slab[:, 1, o : o + n],
                            in0=slab[:, 1, o : o + n],
                            in1=xg_e[:, xoff : xoff + n],
                        )

                    # 9-tap sum on TensorE: identity matmuls accumulate
                    # the product planes into fp32 PSUM
                    acc = pp.tile([C, SL], F32, tag="acc", space="PSUM")
                    n_ft = (n + 511) // 512
                    for j in range(n_ft):
                        f0, f1 = j * 512, min((j + 1) * 512, n)
                        i_t = 0
                        for gname, k0, di in GROUPS:
                            for t in range(KW):
                                nc.tensor.matmul(
                                    acc[:, f0:f1],
                                    ident[:, :],
                                    slabs[gname][:, t, o + f0 : o + f1],
                                    start=(i_t == 0),
                                    stop=(i_t == 3 * KW - 1),
                                )
                                i_t += 1

                    # ScalarE evacuates PSUM -> fp32 staging; store on ACT
                    stg = pg.tile([C, SL], F32, tag="stg")
                    nc.scalar.copy(out=stg[:, 0:n], in_=acc[:, 0:n])
                    nc.scalar.dma_start(
                        out=o_d[:, sr0 * W : sr0 * W + n], in_=stg[:, 0:n]
                    )
                    o += n
                r0 += rr

    return nc


def _split_excess_waits(nc: bass.Bass) -> None:
    """TPB engine instructions carry exactly ONE sync-wait slot; walrus
    refuses instructions with more ("Too many sync wait commands"). Tile's
    sem assignment can emit several waits on one instruction. Split the
    extras onto same-engine NOPs inserted immediately before the
    instruction — the engine sequencer executes them in order, so all
    waits are still satisfied before the instruction runs."""
    import bass_rust

    f = nc.m.functions[0]

    def make_nop(engine):
        ins = nc.engines[engine].nop().ins
        # nop() appends to the currently-open bb; detach it from there
        for bb in f.blocks:
            il = bb.instructions
            for j in range(len(il) - 1, -1, -1):
                if il[j].name == ins.name:
                    del il[j]
                    return ins
        raise AssertionError("freshly created nop not found in any block")

    for bb in f.blocks:
        il = bb.instructions
        i = 0
        while i < len(il):
            ins = il[i]
            si = ins.sync_info
            waits = list(si.on_wait) if si and si.on_wait else []
            if len(waits) > 1:
                updates = list(si.on_update) if si.on_update else []
                ins.sync_info = bass_rust.SyncInfo(
                    on_wait=[waits[-1]], on_update=updates
                )
                for k, w in enumerate(waits[:-1]):
                    nop = make_nop(ins.engine)
                    nop.sync_info = bass_rust.SyncInfo(on_wait=[w], on_update=[])
                    il.insert(i + k, nop)
                i += len(waits) - 1
            i += 1


_NC_CACHE = None


def _get_nc():
    global _NC_CACHE
    if _NC_CACHE is None:
        nc = _build()
        _split_excess_waits(nc)
        _NC_CACHE = nc
    return _NC_CACHE


_RUNNER = None


def _get_runner():
    """Jit the SPMD executable once; repeated kernel() calls reuse it.

    Mirrors concourse.bass2jax.run_bass_via_pjrt's multi-core branch but
    caches the jitted callable (run_bass_via_pjrt builds a fresh closure
    per call, forcing an XLA recompile every time)."""
    global _RUNNER
    if _RUNNER is not None:
        return _RUNNER

    import jax
    from jax.experimental.shard_map import shard_map
    from jax.sharding import Mesh, PartitionSpec

    import concourse.mybir as _mybir
    from concourse import bass2jax

    bass2jax.install_neuronx_cc_hook()
    nc = _get_nc()

    partition_name = (
        nc.partition_id_tensor.name if nc.partition_id_tensor else None
    )
    in_names, out_names, out_avals = [], [], []
    for alloc in nc.m.functions[0].allocations:
        if not isinstance(alloc, _mybir.MemoryLocationSet):
            continue
        name = alloc.memorylocations[0].name
        if alloc.kind == "ExternalInput":
            if name != partition_name:
                in_names.append(name)
        elif alloc.kind == "ExternalOutput":
            out_names.append(name)
            out_avals.append(
                jax.core.ShapedArray(
                    tuple(alloc.tensor_shape), _mybir.dt.np(alloc.dtype)
                )
            )
    n_params = len(in_names)
    n_outs = len(out_names)
    all_in_names = tuple(in_names + out_names)
    if partition_name is not None:
        all_in_names = all_in_names + (partition_name,)
    donate = tuple(range(n_params, n_params + n_outs))

    def _body(*args):
        operands = list(args)
        if partition_name is not None:
            operands.append(bass2jax.partition_id_tensor())
        outs = bass2jax._bass_exec_p.bind(
            *operands,
            out_avals=tuple(out_avals),
            in_names=all_in_names,
            out_names=tuple(out_names),
            lowering_input_output_aliases=(),
            sim_require_finite=True,
            sim_require_nnan=True,
            nc=nc,
        )
        return tuple(outs)

    devices = jax.devices()[:N_CORES]
    mesh = Mesh(np.asarray(devices), ("core",))
    sharded = jax.jit(
        shard_map(
            _body,
            mesh=mesh,
            in_specs=(PartitionSpec("core"),) * (n_params + n_outs),
            out_specs=(PartitionSpec("core"),) * n_outs,
            check_rep=False,
        ),
        donate_argnums=donate,
        keep_unused=True,
    )

    def runner(concat_inputs):
        zeros = [
            np.zeros((N_CORES * a.shape[0], *a.shape[1:]), a.dtype) for a in out_avals
        ]
        outs = sharded(*concat_inputs, *zeros)
        return [np.asarray(o) for o in outs]

    _RUNNER = (runner, in_names, out_names, out_avals)
    return _RUNNER


def prep_inputs(x, conv_weights):
    """Reshape full inputs into the concatenated per-core layout."""
    x = np.ascontiguousarray(np.asarray(x, dtype=np.float32))
    w = np.ascontiguousarray(np.asarray(conv_weights, dtype=np.float32))
    assert x.shape == (N_CORES, C, H, W), x.shape
    assert w.shape == (N_CORES, C * KW * KW, H, W), w.shape
    by_name = {
        "x": x.reshape(N_CORES * C, HW),
        "w": w.reshape(N_CORES * C * KW * KW, HW),
    }
    _, in_names, _, _ = _get_runner()
    return [by_name[n] for n in in_names]


def execute(concat_inputs):
    runner, _, out_names, out_avals = _get_runner()
    outs = runner(concat_inputs)
    i = out_names.index("out")
    return outs[i].reshape(N_CORES, C, H, W)


def kernel(x, conv_weights):
    return execute(prep_inputs(x, conv_weights))


def run(x, conv_weights, **spmd_kwargs):
    """Legacy full-path entry via run_bass_kernel_spmd (no jit caching)."""
    x = np.ascontiguousarray(np.asarray(x, dtype=np.float32))
    w = np.ascontiguousarray(np.asarray(conv_weights, dtype=np.float32))
    n = x.shape[0]
    nc = _get_nc()
    in_maps = [
        {"x": x[i].reshape(C, HW), "w": w[i].reshape(C * KW * KW, HW)}
        for i in range(n)
    ]
    br = run_bass_kernel_spmd(nc, in_maps, core_ids=list(range(n)), **spmd_kwargs)
    out = np.stack([r["out"].reshape(C, H, W) for r in br.results])
    return out, br


# revision 15
# speedup vs baseline: 1.0066x; 1.0066x over previous
"""Involution-style per-pixel depthwise 3x3 conv on 8 trn2 NeuronCores.

out[n,c,h,w] = sum_{k=0..8} w[n,c,k,h,w] * x_pad[n,c,h+k//3,w+k%3]  (pad=1)

Sharding: pure data parallel over N=8 -> one sample per core.
Per core: channels C=128 = SBUF partition dim; free dim = H*W pixels.

v2 design (bf16 compute; harness gate is rel<2e-2, bf16 lands ~1e-3):
- The v1 kernel was DVE-bound: fp32 tensor_tensor runs at 1 elem/cycle,
  so 9 muls + ~4.5 adds per pixel = ~139 us DVE busy vs a ~146 us HBM
  stream (52 MB @ ~358 GB/s).  DMA engines idled 28% waiting on slab
  buffer reuse.
- Weights are cast fp32->bf16 DURING the load DMA (SWDGE/gpsimd path
  does inline dtype conversion; HBM read bytes unchanged, zero engine
  cost).  bf16 tensor_tensor runs at 2 elem/cycle -> products ~50 us.
- DVE 2x mode needs 4B-aligned step-1 operands, but the 3x3 taps read x
  at column offsets -1/0/+1 (odd/even element parity).  So x lives in
  SBUF twice in bf16: xg_even (guard 96) serves the dj=0 taps, xg_odd
  (guard 97) serves dj=+-1; both windows land on even element offsets.
  One [2,2]-strided AP covers the +-1 tap pair in a single DVE op.
- The 9-tap sum runs entirely on the otherwise-idle TensorE: bf16
  identity matmuls accumulate the product planes into fp32 PSUM
  (exact), ScalarE evacuates PSUM->SBUF fp32, HWDGE stores.  No DVE
  adds at all.
- Weight slab border columns (the column-wrap pixels of the dj=+-1
  taps) are zeroed on the VECTOR engine, not gpsimd: a gpsimd memset
  waiting on its slab DMA would stall the gpsimd sequencer and with it
  all later SWDGE descriptor emission -- the weight stream itself.
- One SWDGE queue carries everything that reads HBM: weight chunks of
  (16,16,16,16,16,8,6,2) rows x 3 tap-groups (cast loads, 14 slab
  buffers ~= 4.7 chunks of lookahead) with x cast-loads interleaved
  just-in-time.  The measured stream is gapless at ~26 GB/s per SDMA
  engine.  Stores ride the ACT HWDGE ring.  Tiny final chunks keep the
  exposed mul->matmul->evac->store tail after the last weight byte to
  ~2.5 us; the first chunk's DMAs are emitted before the identity/guard
  setup so the stream starts as early as the ~5.5 us preamble allows.
- Measured: ~139 us best (fully contended HBM reps land 145-165); DVE
  ~48 us busy, PE ~52, ScalarE ~20 -- all far off the critical path.
"""

import numpy as np

import concourse.bass as bass
import concourse.mybir as mybir
from concourse.bass_utils import run_bass_kernel_spmd
from concourse.masks import make_identity
from concourse.tile import TileContext

N_CORES = 8
C, H, W = 128, 96, 96
HW = H * W
KW = 3

F32 = mybir.dt.float32
BF16 = mybir.dt.bfloat16

# row-groups: (name, first tap k0, row shift di)
GROUPS = (("mid", 3, 0), ("top", 0, -1), ("bot", 6, 1))

# guarded x layouts (bf16):
#   xg_even: [96 zeros | x | 96 zeros]  -> dj=0  windows at even offsets
#   xg_odd:  [97 zeros | x | 97 zeros]  -> dj=+-1 windows at even offsets
GE = 96
GO = 97
GXE = HW + 2 * GE
GXO = HW + 2 * GO

# weight DMA row-chunks; each chunk is 3 slab DMAs (one per tap group).
# Small final chunks: the last chunk's mul->matmul->evac->store chain is
# exposed after the final weight byte lands, so keep it tiny.
CHUNKS = (16, 16, 16, 16, 16, 8, 6, 2)
assert sum(CHUNKS) == H
MAXR = max(CHUNKS)
SLAB = MAXR * W          # slab tile elems per tap per partition

# x load chunks (SWDGE cast loads interleaved with the weight chunks)
XCH = 24                 # rows per x chunk
N_XCH = H // XCH

SUBMAX = 16              # compute sub-stripe rows (PSUM sizing)
SL = SUBMAX * W


def _build() -> bass.Bass:
    nc = bass.Bass()
    x_d = nc.dram_tensor("x", [C, HW], F32, kind="ExternalInput")
    w_d = nc.dram_tensor("w", [C * KW * KW, HW], F32, kind="ExternalInput")
    o_d = nc.dram_tensor("out", [C, HW], F32, kind="ExternalOutput")

    w_v = w_d[:].rearrange("(c k) m -> c k m", k=KW * KW)

    with TileContext(nc) as tc:
        with (
            tc.tile_pool(name="px", bufs=1) as px,
            tc.tile_pool(name="pw", bufs=13) as pw,
            tc.tile_pool(name="ps", bufs=2) as ps,
            tc.tile_pool(name="pg", bufs=2) as pg,
            tc.tile_pool(name="pp", bufs=2, space="PSUM") as pp,
        ):
            ident_f = px.tile([C, C], F32)
            ident = px.tile([C, C], BF16)
            xg_e = px.tile([C, GXE], BF16)
            xg_o = px.tile([C, GXO], BF16)

            def load_x_chunk(j):
                """fp32 load of x rows [24j, 24j+24) on the otherwise-idle
                SP HWDGE ring (it starts streaming ~4us before the SWDGE
                queue can emit, so x rides in the startup dead time and
                the SWDGE stream carries only weights).  ScalarE casts the
                chunk into both parity buffers."""
                r0, n = j * XCH, XCH * W
                xs = ps.tile([C, XCH * W], F32, tag="xs", name=f"xs_{j}")
                nc.sync.dma_start(out=xs[:, 0:n], in_=x_d[:, r0 * W : r0 * W + n])
                nc.scalar.copy(
                    out=xg_e[:, GE + r0 * W : GE + r0 * W + n], in_=xs[:, 0:n]
                )
                nc.scalar.copy(
                    out=xg_o[:, GO + r0 * W : GO + r0 * W + n], in_=xs[:, 0:n]
                )

            def slab_pair_ap(slab, o, n):
                """[C, 2, n] AP over slab taps {0,2} at in-slab offset o."""
                base = slab[:, 0, o : o + n]
                ap = [list(p) for p in base.ap]
                return bass.AP(base.tensor, base.offset, [ap[0], [2 * SLAB, 2], [1, n]])

            def x_pair_ap(r0, di, n):
                """[C, 2, n] AP over xg_odd: dj=-1 and dj=+1 windows."""
                off = GO + (r0 + di) * W - 1
                base = xg_o[:, off : off + n]
                ap = [list(p) for p in base.ap]
                return bass.AP(base.tensor, base.offset, [ap[0], [2, 2], [1, n]])

            # x chunk j covers rows [24j, 24j+24); weight chunk ci's bot
            # group reads x rows up to r0+rr (inclusive), so chunk j loads
            # just before the first weight chunk that needs it:
            #   c0 rows 0-15 needs x<=16 -> x0; c1 (16-31) needs 32 -> x1;
            #   c2 (32-47) needs 48 -> x2; c4 (64-79) needs 80 -> x3.
            x_before = {0: 0, 1: 1, 2: 2, 4: 3}

            def issue_chunk_dmas(ci, r0, rr):
                """SWDGE cast loads for one row-chunk (3 tap-group slabs)."""
                cn = rr * W
                slabs = {}
                for gname, k0, di in GROUPS:
                    slab = pw.tile([C, KW, SLAB], BF16, tag="w", name=f"w_{gname}_{ci}")
                    nc.gpsimd.dma_start(
                        out=slab[:, :, 0:cn],
                        in_=w_v[:, k0 : k0 + KW, r0 * W : (r0 + rr) * W],
                    )
                    slabs[gname] = slab
                return slabs

            # startup-critical: get the first x chunk + weight chunk onto
            # the SWDGE queue before the identity/guard setup ops occupy
            # the gpsimd sequencer.
            load_x_chunk(x_before[0])
            slabs0 = issue_chunk_dmas(0, 0, CHUNKS[0])

            make_identity(nc, ident_f)
            nc.vector.tensor_copy(out=ident[:, :], in_=ident_f[:, :])
            # zero the vertical-padding guards once -- on DVE (idle until
            # the first slab lands), keeping the gpsimd queue pure SWDGE
            nc.vector.memset(xg_e[:, 0:GE], 0.0)
            nc.vector.memset(xg_e[:, GE + HW : GXE], 0.0)
            nc.vector.memset(xg_o[:, 0:GO], 0.0)
            nc.vector.memset(xg_o[:, GO + HW : GXO], 0.0)

            r0 = 0
            for ci, rr in enumerate(CHUNKS):
                cn = rr * W
                if ci == 0:
                    slabs = slabs0
                else:
                    if ci in x_before:
                        load_x_chunk(x_before[ci])
                    slabs = issue_chunk_dmas(ci, r0, rr)
                for gname, k0, di in GROUPS:
                    # zero border weight columns on DVE (kills the
                    # column-wrap reads of the dj=+-1 taps)
                    sr = slabs[gname].rearrange("p k (h w) -> p k h w", w=W)
                    nc.vector.memset(sr[:, 0, 0:rr, 0:1], 0.0)
                    nc.vector.memset(sr[:, 2, 0:rr, W - 1 : W], 0.0)

                # compute sub-stripes within this chunk
                o = 0
                while o < cn:
                    n = min(SUBMAX * W, cn - o)
                    sr0 = r0 + o // W
                    for gname, k0, di in GROUPS:
                        slab = slabs[gname]
                        # dj=-1,+1 pair: one 2x-mode TT over [C,2,n]
                        wpair = slab_pair_ap(slab, o, n)
                        nc.vector.tensor_mul(
                            out=wpair, in0=wpair, in1=x_pair_ap(sr0, di, n)
                        )
                        # dj=0 center tap from xg_even
                        xoff = GE + (sr0 + di) * W
                        nc.vector.tensor_mul(
                            out=slab[:, 1, o : o + n],
                            in0=slab[:, 1, o : o + n],
                            in1=xg_e[:, xoff : xoff + n],
                        )

                    # 9-tap sum on TensorE: identity matmuls accumulate
                    # the product planes into fp32 PSUM
                    acc = pp.tile([C, SL], F32, tag="acc", space="PSUM")
                    n_ft = (n + 511) // 512
                    for j in range(n_ft):
                        f0, f1 = j * 512, min((j + 1) * 512, n)
                        i_t = 0
                        for gname, k0, di in GROUPS:
                            for t in range(KW):
                                nc.tensor.matmul(
                                    acc[:, f0:f1],
                                    ident[:, :],
                                    slabs[gname][:, t, o + f0 : o + f1],
                                    start=(i_t == 0),
                                    stop=(i_t == 3 * KW - 1),
                                )
                                i_t += 1

                    # ScalarE evacuates PSUM -> fp32 staging; store on ACT
                    stg = pg.tile([C, SL], F32, tag="stg")
                    nc.scalar.copy(out=stg[:, 0:n], in_=acc[:, 0:n])
                    nc.scalar.dma_start(
                        out=o_d[:, sr0 * W : sr0 * W + n], in_=stg[:, 0:n]
                    )
                    o += n
                r0 += rr

    return nc


def _split_excess_waits(nc: bass.Bass) -> None:
    """TPB engine instructions carry exactly ONE sync-wait slot; walrus
    refuses instructions with more ("Too many sync wait commands"). Tile's
    sem assignment can emit several waits on one instruction. Split the
    extras onto same-engine NOPs inserted immediately before the
    instruction — the engine sequencer executes them in order, so all
    waits are still satisfied before the instruction runs."""
    import bass_rust

    f = nc.m.functions[0]

    def make_nop(engine):
        ins = nc.engines[engine].nop().ins
        # nop() appends to the currently-open bb; detach it from there
        for bb in f.blocks:
            il = bb.instructions
            for j in range(len(il) - 1, -1, -1):
                if il[j].name == ins.name:
                    del il[j]
                    return ins
        raise AssertionError("freshly created nop not found in any block")

    for bb in f.blocks:
        il = bb.instructions
        i = 0
        while i < len(il):
            ins = il[i]
            si = ins.sync_info
            waits = list(si.on_wait) if si and si.on_wait else []
            if len(waits) > 1:
                updates = list(si.on_update) if si.on_update else []
                ins.sync_info = bass_rust.SyncInfo(
                    on_wait=[waits[-1]], on_update=updates
                )
                for k, w in enumerate(waits[:-1]):
                    nop = make_nop(ins.engine)
                    nop.sync_info = bass_rust.SyncInfo(on_wait=[w], on_update=[])
                    il.insert(i + k, nop)
                i += len(waits) - 1
            i += 1


_NC_CACHE = None


def _get_nc():
    global _NC_CACHE
    if _NC_CACHE is None:
        nc = _build()
        _split_excess_waits(nc)
        _NC_CACHE = nc
    return _NC_CACHE


_RUNNER = None


def _get_runner():
    """Jit the SPMD executable once; repeated kernel() calls reuse it.

    Mirrors concourse.bass2jax.run_bass_via_pjrt's multi-core branch but
    caches the jitted callable (run_bass_via_pjrt builds a fresh closure
    per call, forcing an XLA recompile every time)."""
    global _RUNNER
    if _RUNNER is not None:
        return _RUNNER

    import jax
    from jax.experimental.shard_map import shard_map
    from jax.sharding import Mesh, PartitionSpec

    import concourse.mybir as _mybir
    from concourse import bass2jax

    bass2jax.install_neuronx_cc_hook()
    nc = _get_nc()

    partition_name = (
        nc.partition_id_tensor.name if nc.partition_id_tensor else None
    )
    in_names, out_names, out_avals = [], [], []
    for alloc in nc.m.functions[0].allocations:
        if not isinstance(alloc, _mybir.MemoryLocationSet):
            continue
        name = alloc.memorylocations[0].name
        if alloc.kind == "ExternalInput":
            if name != partition_name:
                in_names.append(name)
        elif alloc.kind == "ExternalOutput":
            out_names.append(name)
            out_avals.append(
                jax.core.ShapedArray(
                    tuple(alloc.tensor_shape), _mybir.dt.np(alloc.dtype)
                )
            )
    n_params = len(in_names)
    n_outs = len(out_names)
    all_in_names = tuple(in_names + out_names)
    if partition_name is not None:
        all_in_names = all_in_names + (partition_name,)
    donate = tuple(range(n_params, n_params + n_outs))

    def _body(*args):
        operands = list(args)
        if partition_name is not None:
            operands.append(bass2jax.partition_id_tensor())
        outs = bass2jax._bass_exec_p.bind(
            *operands,
            out_avals=tuple(out_avals),
            in_names=all_in_names,
            out_names=tuple(out_names),
            lowering_input_output_aliases=(),
            sim_require_finite=True,
            sim_require_nnan=True,
            nc=nc,
        )
        return tuple(outs)

    devices = jax.devices()[:N_CORES]
    mesh = Mesh(np.asarray(devices), ("core",))
    sharded = jax.jit(
        shard_map(
            _body,
            mesh=mesh,
            in_specs=(PartitionSpec("core"),) * (n_params + n_outs),
            out_specs=(PartitionSpec("core"),) * n_outs,
            check_rep=False,
        ),
        donate_argnums=donate,
        keep_unused=True,
    )

    def runner(concat_inputs):
        zeros = [
            np.zeros((N_CORES * a.shape[0], *a.shape[1:]), a.dtype) for a in out_avals
        ]
        outs = sharded(*concat_inputs, *zeros)
        return [np.asarray(o) for o in outs]

    _RUNNER = (runner, in_names, out_names, out_avals)
    return _RUNNER


def prep_inputs(x, conv_weights):
    """Reshape full inputs into the concatenated per-core layout."""
    x = np.ascontiguousarray(np.asarray(x, dtype=np.float32))
    w = np.ascontiguousarray(np.asarray(conv_weights, dtype=np.float32))
    assert x.shape == (N_CORES, C, H, W), x.shape
    assert w.shape == (N_CORES, C * KW * KW, H, W), w.shape
    by_name = {
        "x": x.reshape(N_CORES * C, HW),
        "w": w.reshape(N_CORES * C * KW * KW, HW),
    }
    _, in_names, _, _ = _get_runner()
    return [by_name[n] for n in in_names]


def execute(concat_inputs):
    runner, _, out_names, out_avals = _get_runner()
    outs = runner(concat_inputs)
    i = out_names.index("out")
    return outs[i].reshape(N_CORES, C, H, W)


def kernel(x, conv_weights):
    return execute(prep_inputs(x, conv_weights))


def run(x, conv_weights, **spmd_kwargs):
    """Legacy full-path entry via run_bass_kernel_spmd (no jit caching)."""
    x = np.ascontiguousarray(np.asarray(x, dtype=np.float32))
    w = np.ascontiguousarray(np.asarray(conv_weights, dtype=np.float32))
    n = x.shape[0]
    nc = _get_nc()
    in_maps = [
        {"x": x[i].reshape(C, HW), "w": w[i].reshape(C * KW * KW, HW)}
        for i in range(n)
    ]
    br = run_bass_kernel_spmd(nc, in_maps, core_ids=list(range(n)), **spmd_kwargs)
    out = np.stack([r["out"].reshape(C, H, W) for r in br.results])
    return out, br
